# revision 16
# baseline (speedup 1.0000x reference)
"""Trainium2 Bass kernel for nn_Net_67259187855904 (DynParasLSTM + ExpHYDRO).

Sharding: basin dim N=1024 split across 8 cores (128 basins/core), pure data
parallel; weights replicated. Full inputs in, full output out.
"""
import numpy as np
from contextlib import ExitStack

import concourse.bass as bass
import concourse.mybir as mybir
from concourse import tile
from concourse.bass_utils import run_bass_kernel_spmd
from concourse.bacc import Bacc

f32 = mybir.dt.float32
f32r = mybir.dt.float32r
ALU = mybir.AluOpType
ACTF = mybir.ActivationFunctionType

N_CORES = 8
NB = 128           # basins per core
L_FULL = 1095
F_IN = 32
H = 256
OUT = 10
TMAX_UH = 15
TLEN = 5
EPS = 0.0003
TINY = 1e-30
NAN = float("nan")

LANCZOS = [0.99999999999980993, 676.5203681218851, -1259.1392167224028,
           771.32342877765313, -176.61502916214059, 12.507343278686905,
           -0.13857109526572012, 9.9843695780195716e-6, 1.5056327351493116e-7]


def r32(ap):
    return ap.bitcast(f32r)


def build_nc(L, has_gbias):
    nc = Bacc()

    # ---- DRAM I/O (per-core shapes) ----
    # const pack (f32): P | T | PET | eye(128) | lnt(15) | tg(15) | bin(2) |
    #                   bout | sc | off (1 col each, rows 0:10) [| gbias 4H]
    CW = 3 * L + 128 + 15 + 15 + 2 + 3 + (4 * H if has_gbias else 0)
    # weight pack (f32r): WihT(2*4H) | WhhT(2*4H) | Wout2(2*OUT) | Win(2*128, rows 0:32)
    WW = 2 * 4 * H + 2 * 4 * H + 2 * OUT + 256
    x_d = nc.dram_tensor("x", [NB, L * F_IN], f32, kind="ExternalInput")
    cp_d = nc.dram_tensor("cpack", [128, CW], f32, kind="ExternalInput")
    wp_d = nc.dram_tensor("wpack", [128, WW], f32r, kind="ExternalInput")
    out_d = nc.dram_tensor("out_q", [NB, L], f32, kind="ExternalOutput")

    GH = 2 * H  # 512, one psum half of the 4H gates

    with tile.TileContext(nc) as tc, ExitStack() as big:
        # ---------- persistent SBUF ----------
        pers = big.enter_context(tc.tile_pool(name="pers", bufs=1))
        dram = big.enter_context(tc.tile_pool(name="dram", bufs=1, space="DRAM"))

        cp = pers.tile([128, CW], f32, tag="cp")
        nc.sync.dma_start(cp[:], cp_d[:])
        wp = pers.tile([128, WW], f32r, tag="wp")
        nc.sync.dma_start(wp[:], wp_d[:])

        P_sb = cp[:, 0:L]
        T_sb = cp[:, L:2 * L]
        PET_sb = cp[:, 2 * L:3 * L]
        c0 = 3 * L
        eye = cp[:, c0:c0 + 128]
        lnt_sb = cp[:, c0 + 128:c0 + 143]
        tg_sb = cp[:, c0 + 143:c0 + 158]
        bin_sb = cp[:, c0 + 158:c0 + 160]
        bout_sb = cp[0:OUT, c0 + 160:c0 + 161]
        sc_sb = cp[0:OUT, c0 + 161:c0 + 162]
        off_sb = cp[0:OUT, c0 + 162:c0 + 163]
        gb_sb = cp[:, c0 + 163:c0 + 163 + 4 * H] if has_gbias else None

        Wih_sb = wp[:, 0:2 * 4 * H]
        Whh_sb = wp[:, 2 * 4 * H:4 * 4 * H]
        Wout_sb = wp[:, 4 * 4 * H:4 * 4 * H + 2 * OUT]
        wn0 = 4 * 4 * H + 2 * OUT
        Win_sb = wp[0:F_IN, wn0:wn0 + 256]

        par_nt = pers.tile([128, L * OUT], f32, tag="par_nt")   # [n, t*10+j]
        Q_sb = pers.tile([128, L], f32, tag="Q_sb")
        SmS_st = pers.tile([128, L], f32, tag="SmS_st")

        hT_dram = [dram.tile([128, L * 128], f32r, tag=f"hT{j}", name=f"hT{j}") for j in range(2)]

        # =========================================================
        # Phase A: fcIn  -> h_inT (k-tile-major) in DRAM
        # =========================================================
        CH_A = 4  # timesteps per chunk (free 512)
        n_chA = (L + CH_A - 1) // CH_A
        with tc.tile_pool(name="xa", bufs=3) as xa, \
             tc.tile_pool(name="xt_ps", bufs=2, space="PSUM") as xt_ps, \
             tc.tile_pool(name="hr_ps", bufs=2, space="PSUM") as hr_ps, \
             tc.tile_pool(name="hA", bufs=3) as hA:
            for ci in range(n_chA):
                t0 = ci * CH_A
                nt = min(CH_A, L - t0)
                xtile = xa.tile([NB, CH_A * F_IN], f32, tag="xtile")
                nc.sync.dma_start(xtile[:, :nt * F_IN],
                                  x_d[:, t0 * F_IN:(t0 + nt) * F_IN])
                xps = xt_ps.tile([F_IN, CH_A * 128], f32, tag="xps")
                for k in range(nt):
                    nc.tensor.transpose(xps[:, k * 128:(k + 1) * 128],
                                        xtile[:, k * F_IN:(k + 1) * F_IN], eye)
                xT = xa.tile([F_IN, CH_A * 128], f32r, tag="xT")
                nc.vector.tensor_copy(xT[:, :nt * 128], xps[:, :nt * 128])
                for j in range(2):
                    hps = hr_ps.tile([128, CH_A * 128], f32, tag=f"hps{j}")
                    nc.tensor.matmul(hps[:, :nt * 128],
                                     Win_sb[:, j * 128:(j + 1) * 128],
                                     r32(xT[:, :nt * 128]), start=True, stop=True)
                    hsb = hA.tile([128, CH_A * 128], f32r, tag=f"hsb{j}")
                    nc.scalar.activation(hsb[:, :nt * 128], hps[:, :nt * 128],
                                         ACTF.Relu, bias=bin_sb[:, j:j + 1])
                    nc.sync.dma_start(hT_dram[j][:, t0 * 128:(t0 + nt) * 128],
                                      hsb[:, :nt * 128])

        # =========================================================
        # Phase B: LSTM scan + fcOut + par transpose
        # =========================================================
        CH_B = 16   # h_inT chunk (timesteps per DMA)
        n_chB = (L + CH_B - 1) // CH_B
        with tc.tile_pool(name="hin", bufs=2) as hinp, \
             tc.tile_pool(name="gps", bufs=2, space="PSUM") as gps, \
             tc.tile_pool(name="trp", bufs=1, space="PSUM") as trp, \
             tc.tile_pool(name="pps", bufs=1, space="PSUM") as pps, \
             tc.tile_pool(name="ptr", bufs=1, space="PSUM") as ptr, \
             tc.tile_pool(name="cell", bufs=2) as cell, \
             tc.tile_pool(name="hTb", bufs=2) as hTbp, \
             tc.tile_pool(name="st", bufs=2) as stp:

            c_st = stp.tile([128, H], f32, tag="c_st")
            nc.vector.memset(c_st[:], 0.0)
            hT_prev = None  # [2 tiles of [128,128]] from previous step; None => h=0

            hin_tiles = None
            hTbuf = None
            for t in range(L):
                if t % CH_B == 0:
                    nt = min(CH_B, L - t)
                    hin_tiles = [hinp.tile([128, CH_B * 128], f32r, tag=f"hin{j}", name=f"hin{j}")
                                 for j in range(2)]
                    for j in range(2):
                        nc.sync.dma_start(hin_tiles[j][:, :nt * 128],
                                          hT_dram[j][:, t * 128:(t + nt) * 128])
                tc_i = t % CH_B
                tq = t % 4
                if tq == 0:
                    hTbuf = [hTbp.tile([128, 512], f32r, tag=f"hTb{j}", name=f"hTb{j}")
                             for j in range(2)]

                # gates psum: two halves of 512
                ph = [gps.tile([128, GH], f32, tag=f"gps{gh}", name=f"gps{gh}") for gh in range(2)]
                for gh in range(2):
                    g0 = gh * GH
                    for j in range(2):
                        nc.tensor.matmul(
                            ph[gh][:],
                            r32(hin_tiles[j][:, tc_i * 128:(tc_i + 1) * 128]),
                            Wih_sb[:, j * 4 * H + g0:j * 4 * H + g0 + GH],
                            start=(j == 0), stop=(j == 1 and hT_prev is None))
                    if hT_prev is not None:
                        for j in range(2):
                            nc.tensor.matmul(
                                ph[gh][:],
                                r32(hT_prev[j]),
                                Whh_sb[:, j * 4 * H + g0:j * 4 * H + g0 + GH],
                                start=False, stop=(j == 1))

                # activations: gates order i | f | g | o
                gin = ph
                if has_gbias:
                    gsb = cell.tile([128, 4 * H], f32, tag="gsb")
                    for gh in range(2):
                        nc.vector.tensor_add(gsb[:, gh * GH:(gh + 1) * GH], ph[gh][:],
                                             gb_sb[:, gh * GH:(gh + 1) * GH])
                    gin = [gsb[:, 0:GH], gsb[:, GH:2 * GH]]
                sig_if = cell.tile([128, 512], f32, tag="sig_if")
                nc.scalar.activation(sig_if[:], gin[0][:], ACTF.Sigmoid)
                tanh_g = cell.tile([128, 256], f32, tag="tanh_g")
                nc.scalar.activation(tanh_g[:], gin[1][:, 0:256], ACTF.Tanh)
                sig_o = cell.tile([128, 256], f32, tag="sig_o")
                nc.scalar.activation(sig_o[:], gin[1][:, 256:512], ACTF.Sigmoid)

                # cell math
                m1 = cell.tile([128, 256], f32, tag="m1")
                nc.vector.tensor_mul(m1[:], sig_if[:, 256:512], c_st[:])
                m2 = cell.tile([128, 256], f32, tag="m2")
                nc.vector.tensor_mul(m2[:], sig_if[:, 0:256], tanh_g[:])
                c_new = stp.tile([128, H], f32, tag="c_st")
                nc.vector.tensor_add(c_new[:], m1[:], m2[:])
                c_st = c_new
                tc_t = cell.tile([128, 256], f32, tag="tc_t")
                nc.scalar.activation(tc_t[:], c_new[:], ACTF.Tanh)
                h_new = cell.tile([128, 256], f32, tag="h_new")
                nc.vector.tensor_mul(h_new[:], sig_o[:], tc_t[:])

                # transpose h -> hTbuf slots
                hT_prev = []
                for j in range(2):
                    tp = trp.tile([128, 128], f32, tag=f"trp{j}")
                    nc.tensor.transpose(tp[:], h_new[:, j * 128:(j + 1) * 128], eye)
                    dst = hTbuf[j][:, tq * 128:(tq + 1) * 128]
                    nc.vector.tensor_copy(dst, tp[:])
                    hT_prev.append(dst)

                # fcOut every 4 steps
                if tq == 3 or t == L - 1:
                    nt4 = tq + 1
                    pp = pps.tile([OUT, 512], f32, tag="pp")
                    for j in range(2):
                        nc.tensor.matmul(pp[:, :nt4 * 128],
                                         Wout_sb[:, j * OUT:(j + 1) * OUT],
                                         r32(hTbuf[j][:, :nt4 * 128]),
                                         start=(j == 0), stop=(j == 1))
                    p_sb = cell.tile([OUT, 512], f32, tag="p_sb")
                    nc.scalar.activation(p_sb[:, :nt4 * 128], pp[:, :nt4 * 128],
                                         ACTF.Sigmoid, bias=bout_sb)
                    nc.vector.tensor_scalar(p_sb[:, :nt4 * 128], p_sb[:, :nt4 * 128],
                                            sc_sb, off_sb,
                                            ALU.mult, ALU.add)
                    pt = ptr.tile([128, 4 * OUT], f32, tag="pt")
                    t_base = t - nt4 + 1
                    for k in range(nt4):
                        nc.tensor.transpose(pt[:, k * OUT:(k + 1) * OUT],
                                            p_sb[:, k * 128:(k + 1) * 128],
                                            eye[0:OUT, 0:OUT])
                    nc.vector.tensor_copy(
                        par_nt[:, t_base * OUT:(t_base + nt4) * OUT],
                        pt[:, :nt4 * OUT])

        # =========================================================
        # Phase C: vectorized stream precompute
        # =========================================================
        def pv(j, t0=0, tn=None):
            """strided par view: param j over timesteps [t0, t0+tn) -> [128, tn]"""
            tn = L - t0 if tn is None else tn
            if tn == 1:
                k = t0 * OUT + j
                return par_nt[:, k:k + 1]
            return par_nt[:][:, t0 * OUT + j::OUT][:, 0:tn]

        with tc.tile_pool(name="str", bufs=1) as sp:
            zL = sp.tile([128, L], f32, tag="zL")
            nc.vector.memset(zL[:], 0.0)
            z1 = sp.tile([128, 1], f32, tag="z1")
            nc.vector.memset(z1[:], 0.0)
            tiny1 = sp.tile([128, 1], f32, tag="tiny1")
            nc.vector.memset(tiny1[:], TINY)
            eps1 = sp.tile([128, 1], f32, tag="eps1")
            nc.vector.memset(eps1[:], EPS)

            ps_s = sp.tile([128, L], f32, tag="ps_s")
            pm_s = sp.tile([128, L], f32, tag="pm_s")
            fz_s = sp.tile([128, L], f32, tag="fz_s")
            f1a_s = sp.tile([128, L], f32, tag="f1a_s")
            se_s = sp.tile([128, L], f32, tag="se_s")   # smax - EPS
            eb_s = sp.tile([128, L], f32, tag="eb_s")   # ln(qmax) - fpar*smax
            w1 = sp.tile([128, L], f32, tag="w1")
            w2 = sp.tile([128, L], f32, tag="w2")

            # cold mask & ps/pr (pr kept in w2)
            nc.vector.tensor_tensor(ps_s[:], T_sb[:], pv(0), ALU.is_lt)
            nc.vector.tensor_mul(ps_s[:], ps_s[:], P_sb[:])          # ps
            nc.vector.tensor_sub(w2[:], P_sb[:], ps_s[:])            # pr
            # melt_pos = relu((T - tmx) * ddf)
            nc.vector.tensor_sub(w1[:], T_sb[:], pv(2))
            nc.vector.tensor_mul(w1[:], w1[:], pv(1))
            nc.vector.tensor_scalar(w1[:], w1[:], 0.0, None, ALU.max)  # mp
            # d0 = mp - shift(ps);   (reuse w1 in place for d0)
            nc.vector.tensor_sub(w1[:, 1:L], w1[:, 1:L], ps_s[:, 0:L - 1])
            # r scan -> pm_s as scratch
            nc.vector.tensor_tensor_scan(pm_s[:], w1[:], zL[:], -0.001,
                                         ALU.add, ALU.min)
            # Sw = ps - r   (w1 <- Sw)
            nc.vector.tensor_sub(w1[:], ps_s[:], pm_s[:])
            # pm = pr + ps - Sw + shift(Sw) (+0.001 at t0)
            nc.vector.tensor_add(pm_s[:], w2[:], ps_s[:])
            nc.vector.tensor_sub(pm_s[:], pm_s[:], w1[:])
            nc.vector.tensor_add(pm_s[:, 1:L], pm_s[:, 1:L], w1[:, 0:L - 1])
            nc.vector.tensor_scalar(pm_s[:, 0:1], pm_s[:, 0:1], 0.001, None, ALU.add)
            # freeze: rolling 5-sum of T < 0
            nc.vector.tensor_copy(w1[:], T_sb[:])
            for k in range(1, TLEN):
                nc.vector.tensor_add(w1[:, k:L], w1[:, k:L], T_sb[:, 0:L - k])
            nc.vector.tensor_tensor(fz_s[:], w1[:], zL[:], ALU.is_lt)
            # f1a = freeze * (1 - alpha) = freeze - freeze*alpha
            nc.vector.tensor_mul(f1a_s[:], fz_s[:], pv(6))
            nc.vector.tensor_sub(f1a_s[:], fz_s[:], f1a_s[:])
            # smax-EPS
            nc.vector.tensor_scalar(se_s[:], pv(3), EPS, None, ALU.subtract)
            # expbias = ln(qmax) - fpar*smax
            nc.scalar.activation(eb_s[:], pv(5), ACTF.Ln)
            nc.vector.tensor_mul(w1[:], pv(4), pv(3))
            nc.vector.tensor_sub(eb_s[:], eb_s[:], w1[:])

            # =====================================================
            # Phase D: soil scan
            # =====================================================
            with tc.tile_pool(name="soil", bufs=2) as so, \
                 tc.tile_pool(name="soil_st", bufs=2) as sost:
                Ssl = sost.tile([128, 1], f32, tag="Ssl")
                Sss = sost.tile([128, 1], f32, tag="Sss")
                nc.vector.memset(Ssl[:], 0.001)
                nc.vector.memset(Sss[:], 0.001)
                stt = nc.vector.scalar_tensor_tensor
                for t in range(L):
                    col = slice(t, t + 1)
                    Ss = so.tile([128, 1], f32, tag="Ss")
                    stt(Ss[:], Ssl[:], Sss[:, 0:1], se_s[:, col], ALU.add, ALU.min)
                    dd = so.tile([128, 1], f32, tag="dd")
                    stt(dd[:], Ss[:], Sss[:, 0:1], fz_s[:, col], ALU.subtract, ALU.mult)
                    aX = so.tile([128, 1], f32, tag="aX")
                    stt(aX[:], dd[:], Sss[:, 0:1], pv(6, t, 1), ALU.add, ALU.mult)
                    Sss2 = sost.tile([128, 1], f32, tag="Sss")
                    stt(Sss2[:], Sss[:], f1a_s[:, col], aX[:], ALU.mult, ALU.add)
                    Sss = Sss2
                    Ssl2 = sost.tile([128, 1], f32, tag="Ssl")
                    stt(Ssl2[:], Ss[:], Sss[:, 0:1], eps1[:], ALU.subtract, ALU.max)
                    Ssl = Ssl2
                    # SmS (real) into stream; SmS_pos for log
                    lg = so.tile([128, 2], f32, tag="lg")
                    stt(SmS_st[:, col], pv(3, t, 1), Sss[:, 0:1], z1[:], ALU.subtract, ALU.add)
                    stt(lg[:, 1:2], pv(3, t, 1), Sss[:, 0:1], tiny1[:], ALU.subtract, ALU.max)
                    nc.vector.tensor_copy(lg[:, 0:1], Ssl[:])
                    lns = so.tile([128, 2], f32, tag="lns")
                    nc.scalar.activation(lns[:], lg[:], ACTF.Ln)
                    ex = so.tile([128, 2], f32, tag="ex")
                    stt(ex[:, 0:1], lns[:, 0:1], lns[:, 1:2], pv(7, t, 1),
                        ALU.subtract, ALU.mult)
                    tq_ = so.tile([128, 1], f32, tag="tq_")
                    stt(tq_[:], Ssl[:], Sss[:, 0:1], pv(4, t, 1), ALU.add, ALU.mult)
                    stt(ex[:, 1:2], tq_[:], eb_s[:, col], z1[:], ALU.add, ALU.add)
                    eo = so.tile([128, 2], f32, tag="eo")
                    nc.scalar.activation(eo[:], ex[:], ACTF.Exp)
                    capQ = so.tile([128, 1], f32, tag="capQ")
                    stt(capQ[:], Ssl[:], EPS, pv(5, t, 1), ALU.subtract, ALU.max)
                    Qb = so.tile([128, 1], f32, tag="Qb")
                    nc.vector.tensor_tensor(Qb[:], eo[:, 1:2], capQ[:], ALU.min)
                    etr = so.tile([128, 1], f32, tag="etr")
                    stt(etr[:], eo[:, 0:1], PET_sb[:, col], PET_sb[:, col],
                        ALU.mult, ALU.min)
                    u = so.tile([128, 1], f32, tag="u")
                    stt(u[:], Ssl[:], Qb[:, 0:1], etr[:], ALU.subtract, ALU.subtract)
                    nc.vector.tensor_scalar(u[:], u[:], 0.0, None, ALU.max)
                    upm = so.tile([128, 1], f32, tag="upm")
                    stt(upm[:], u[:], pm_s[:, col], z1[:], ALU.add, ALU.add)
                    Ssl3 = sost.tile([128, 1], f32, tag="Ssl")
                    nc.vector.tensor_tensor(Ssl3[:], upm[:], SmS_st[:, col], ALU.min)
                    Ssl = Ssl3
                    stt(Q_sb[:, col], upm[:], Ssl[:, 0:1], Qb[:], ALU.subtract, ALU.add)

            # =====================================================
            # Phase E: poison mask, UH weights, conv, output
            # =====================================================
            nan1 = sp.tile([128, L], f32, tag="nanL")
            nc.vector.memset(nan1[:], NAN)
            nc.vector.tensor_tensor(w1[:], SmS_st[:], zL[:], ALU.is_lt)
            nc.vector.tensor_tensor_scan(w2[:], w1[:], zL[:], 0.0, ALU.max, ALU.add)
            nc.vector.copy_predicated(Q_sb[:], w2[:].bitcast(mybir.dt.uint32), nan1[:])

            with tc.tile_pool(name="uh", bufs=1) as uh:
                aa = uh.tile([128, TMAX_UH], f32, tag="aa")
                th = uh.tile([128, TMAX_UH], f32, tag="th")
                # aa = relu(parA[:15 over t]) + 0.1 ; th = relu(parB) + 0.5
                nc.vector.tensor_scalar(aa[:], pv(8, 0, TMAX_UH), 0.0, 0.1, ALU.max, ALU.add)
                nc.vector.tensor_scalar(th[:], pv(9, 0, TMAX_UH), 0.0, 0.5, ALU.max, ALU.add)
                A = uh.tile([128, TMAX_UH], f32, tag="A")
                nc.vector.memset(A[:], LANCZOS[0])
                rc = uh.tile([128, TMAX_UH], f32, tag="rc")
                t1 = uh.tile([128, TMAX_UH], f32, tag="t1")
                for k in range(1, 9):
                    nc.vector.tensor_scalar(t1[:], aa[:], float(k - 1.0), None, ALU.add)
                    nc.vector.reciprocal(rc[:], t1[:])
                    nc.vector.tensor_scalar(rc[:], rc[:], float(LANCZOS[k]), None, ALU.mult)
                    nc.vector.tensor_add(A[:], A[:], rc[:])
                # tl = aa + 6.5 ; pack [A, tl, th] logs
                pk = uh.tile([128, 3 * TMAX_UH], f32, tag="pk")
                nc.vector.tensor_copy(pk[:, 0:TMAX_UH], A[:])
                nc.vector.tensor_scalar(pk[:, TMAX_UH:2 * TMAX_UH], aa[:], 6.5, None, ALU.add)
                nc.vector.tensor_copy(pk[:, 2 * TMAX_UH:3 * TMAX_UH], th[:])
                lpk = uh.tile([128, 3 * TMAX_UH], f32, tag="lpk")
                nc.scalar.activation(lpk[:], pk[:], ACTF.Ln)
                # lgam = 0.9189385 + (aa-0.5)*ln(tl) - tl + ln(A)
                lga = uh.tile([128, TMAX_UH], f32, tag="lga")
                nc.vector.tensor_scalar(lga[:], aa[:], 0.5, None, ALU.subtract)
                nc.vector.tensor_mul(lga[:], lga[:], lpk[:, TMAX_UH:2 * TMAX_UH])
                nc.vector.tensor_sub(lga[:], lga[:], pk[:, TMAX_UH:2 * TMAX_UH])
                nc.vector.tensor_add(lga[:], lga[:], lpk[:, 0:TMAX_UH])
                nc.vector.tensor_scalar(lga[:], lga[:], 0.91893853320467274, None, ALU.add)
                # w_arg = (aa-1)*lnt - tg/th - lgam - aa*ln(th)
                wa = uh.tile([128, TMAX_UH], f32, tag="wa")
                nc.vector.tensor_scalar(wa[:], aa[:], 1.0, None, ALU.subtract)
                nc.vector.tensor_mul(wa[:], wa[:], lnt_sb)
                nc.vector.reciprocal(rc[:], th[:])
                nc.vector.tensor_mul(rc[:], rc[:], tg_sb)
                nc.vector.tensor_sub(wa[:], wa[:], rc[:])
                nc.vector.tensor_sub(wa[:], wa[:], lga[:])
                nc.vector.tensor_mul(t1[:], aa[:], lpk[:, 2 * TMAX_UH:3 * TMAX_UH])
                nc.vector.tensor_sub(wa[:], wa[:], t1[:])
                wp = uh.tile([128, TMAX_UH], f32, tag="wp")
                nc.scalar.activation(wp[:], wa[:], ACTF.Exp)
                ssum = uh.tile([128, 1], f32, tag="ssum")
                nc.vector.tensor_reduce(ssum[:], wp[:], mybir.AxisListType.X, ALU.add)
                nc.vector.reciprocal(ssum[:], ssum[:])
                nc.vector.tensor_scalar(wp[:], wp[:], ssum[:, 0:1], None, ALU.mult)

                # conv: Qr = sum_k wp[:,k] * shift(Q, k)
                Qr = sp.tile([128, L], f32, tag="QrL")
                nc.vector.tensor_scalar(Qr[:], Q_sb[:], wp[:, 0:1], None, ALU.mult)
                for k in range(1, TMAX_UH):
                    nc.vector.scalar_tensor_tensor(
                        Qr[:, k:L], Q_sb[:, 0:L - k], wp[:, k:k + 1], Qr[:, k:L],
                        ALU.mult, ALU.add)
                nc.sync.dma_start(out_d[:], Qr[:])

    return nc


# ---------------- host side ----------------
_NC_CACHE = {}


def _get_nc(L, has_gbias):
    key = (L, has_gbias)
    if key not in _NC_CACHE:
        nc = build_nc(L, has_gbias)
        nc.finalize()
        _NC_CACHE[key] = nc
    return _NC_CACHE[key]


def _host_inputs(inputs, L):
    f = np.float32
    b_in = np.ascontiguousarray(inputs["b_in"], f)
    W_ih = np.ascontiguousarray(inputs["W_ih"], f)
    W_hh = np.ascontiguousarray(inputs["W_hh"], f)
    b_ih = np.ascontiguousarray(inputs["b_ih"], f)
    b_hh = np.ascontiguousarray(inputs["b_hh"], f)
    W_out = np.ascontiguousarray(inputs["W_out"], f)
    b_out = np.ascontiguousarray(inputs["b_out"], f)
    W_in = np.ascontiguousarray(inputs["W_in"], f)

    gbias = (b_ih + b_hh).astype(f)
    has_gbias = bool(np.any(gbias != 0.0))

    # weight pack [128, WW] (f32r bits == f32 bits)
    WihT = np.ascontiguousarray(W_ih.T)
    WhhT = np.ascontiguousarray(W_hh.T)
    wih2 = np.concatenate([WihT[0:128], WihT[128:256]], axis=1)      # [128, 2048]
    whh2 = np.concatenate([WhhT[0:128], WhhT[128:256]], axis=1)
    wout2 = np.concatenate([W_out[0:128], W_out[128:256]], axis=1)   # [128, 20]
    win_pad = np.zeros((128, 256), f)
    win_pad[0:F_IN] = W_in                                           # rows 0:32
    wpack = np.concatenate([wih2, whh2, wout2, win_pad], axis=1)

    # const pack [128, CW]
    tgrid = np.arange(0.5, float(TMAX_UH), 1.0, dtype=f)
    col10 = lambda v: np.pad(v.reshape(-1, 1), ((0, 118), (0, 0))).astype(f)
    sc = np.array([-3, 5, 3, 1400, 0.1, 40, 1, 1, 2.9, 6.5], f)
    off = np.array([0, 0, 0, 100, 0, 10, 0, 1, 0, 0], f)
    const_tail = [np.eye(128, dtype=f),
                  np.broadcast_to(np.log(tgrid)[None, :], (128, TMAX_UH)),
                  np.broadcast_to(tgrid[None, :], (128, TMAX_UH)),
                  np.ascontiguousarray(b_in.reshape(2, 128).T),
                  col10(b_out), col10(sc), col10(off)]
    if has_gbias:
        const_tail.append(np.broadcast_to(gbias[None, :], (128, 4 * H)))

    x = np.ascontiguousarray(inputs["x"], f)
    P = np.ascontiguousarray(inputs["P"], f)
    T = np.ascontiguousarray(inputs["T"], f)
    PET = np.ascontiguousarray(inputs["PET"], f)
    in_maps = []
    for ci in range(N_CORES):
        s = slice(ci * NB, (ci + 1) * NB)
        cpack = np.concatenate([P[s], T[s], PET[s]] + const_tail, axis=1)
        m = {
            "x": np.ascontiguousarray(x[s].reshape(NB, L * F_IN)),
            "cpack": np.ascontiguousarray(cpack, dtype=f),
            "wpack": wpack,
        }
        in_maps.append(m)
    return in_maps, has_gbias


def kernel(**inputs) -> np.ndarray:
    x = inputs["x"]
    n, L, _ = x.shape
    assert n == N_CORES * NB
    in_maps, has_gbias = _host_inputs(inputs, L)
    nc = _get_nc(L, has_gbias)
    res = run_bass_kernel_spmd(nc, in_maps, core_ids=list(range(N_CORES)))
    outs = res.results
    return np.concatenate([outs[ci]["out_q"] for ci in range(N_CORES)], axis=0)
